# revision 20
# baseline (speedup 1.0000x reference)
"""Trainium2 Bass kernel for a separable 3D Haar DWT (nn_DWT3D).

Problem: x [2, 128, 128, 128, 4] fp32, A [128, 128] (orthonormal Haar
analysis filter bank, 2-tap stride-2). Output: subband concat
[2, 64, 64, 64, 32].

Strategy (8 NeuronCores):
- Data-parallel over (batch, channel): B*C = 8 independent [128,128,128]
  volume transforms, one per core. Host deinterleaves channels on the
  way in and assembles the subband concat on the way out (numpy).
- KEY TRICK: the host lays each volume out with partition index
  p = (i&1, j&1, k&1, (k>>1)&15) and free index f = (i>>1, j>>1, k>>5).
  ALL THREE Haar butterfly passes then act on the partition axis, so ONE
  128-wide PE matmul (butterfly_i (x) butterfly_j (x) butterfly_k (x)
  I_16, 8 nonzeros per row, h^3 folded in) performs the whole 3D
  transform. No vector-engine pass at all.
- fp16 everywhere off-chip (tolerance is 2e-2; fp16 keeps us ~1e-3):
  halves DMA bytes and runs the PE at 1 cycle/row instead of fp32's
  effective 8 (2 half-speed passes).
- The reference's subband concat never emits the (a=H, b=L, d=H) octant
  (its HLH slice equals HHH), so only 112 of 128 output rows are
  computed, drained, and DMA'd out.
- Per 2048-col chunk: SP issues in-DMA -> 4 PE matmuls (512 cols each,
  one PSUM bank) -> drain PSUM->SBUF fp16 split across ACT and DVE ->
  GpSimd (SWDGE) issues out-DMA.
"""

import numpy as np

_N = 128
_CORES = 8
_F = _N * _N  # 16384 free columns per volume
# chunk widths; middle chunks give 4KB DMA row-descriptors, small edges
# shorten pipeline fill and drain. DMA count must stay low (~20): the DMA
# semaphore pool is small and recycles in issue order.
_CHUNKS = [1024] + [2048] * 7 + [1024]
_QROWS = 112  # 7 of 8 output octants * 16 kh rows

# Octants (lhi, lhj, lhk) in device output row order; (1, 0, 1) omitted.
_OCT = [(0, 0, 0), (1, 0, 0), (0, 1, 0), (1, 1, 0),
        (0, 0, 1), (0, 1, 1), (1, 1, 1)]
# reference band order LLL,LLH,LHL,LHH,HLL,HLH,HHL,HHH -> index into _OCT
_OMAP = [0, 1, 2, 3, 4, 6, 5, 6]

_cache = {}


def _build():
    import concourse.mybir as mybir
    from concourse import bacc
    from concourse.tile import TileContext

    nc = bacc.Bacc("TRN2", target_bir_lowering=False, debug=False,
                   num_devices=_CORES)
    f16 = mybir.dt.float16
    f32 = mybir.dt.float32
    v = nc.dram_tensor("v", [_N, _F], f16, kind="ExternalInput")
    w = nc.dram_tensor("w", [_N, _QROWS], f16, kind="ExternalInput")
    # two output regions: ACT-drained (even 512-col banks) and DVE-drained
    # (odd banks); host re-interleaves
    ya_d = nc.dram_tensor("ya", [_QROWS, _F // 2], f16, kind="ExternalOutput")
    yb_d = nc.dram_tensor("yb", [_QROWS, _F // 2], f16, kind="ExternalOutput")

    with TileContext(nc) as tc:
        with (
            tc.tile_pool(name="wpool", bufs=1) as wpool,
            tc.tile_pool(name="vin", bufs=1) as vpool,
            tc.tile_pool(name="ya", bufs=1) as yapool,
            tc.tile_pool(name="yb", bufs=1) as ybpool,
            tc.tile_pool(name="psa", bufs=2, space="PSUM") as psapool,
            tc.tile_pool(name="psb", bufs=2, space="PSUM") as psbpool,
        ):
            wt = wpool.tile([_N, _QROWS], f16)
            nc.scalar.dma_start(out=wt[:], in_=w[:])

            vin = vpool.tile([_N, _F], f16)
            yat = yapool.tile([_QROWS, _F // 2], f16)
            ybt = ybpool.tile([_QROWS, _F // 2], f16)

            offs = np.cumsum([0] + _CHUNKS)
            # the first 5 input DMAs go out on Sync before anything else;
            # the rest issue from Scalar between drains so their descriptors
            # enqueue AFTER the early output groups (the DMA engines service
            # descriptors roughly FIFO, so a deep input backlog delays
            # output start)
            for k in range(5):
                nc.sync.dma_start(
                    out=vin[:, offs[k]:offs[k + 1]],
                    in_=v[:, offs[k]:offs[k + 1]])

            h = 0     # per-side output col offset
            hmark = 0  # start of the not-yet-DMA'd output region
            for k, cw in enumerate(_CHUNKS):
                off = offs[k]
                ha = cw // 2

                # independent PSUM tiles for the ACT- and DVE-drained halves
                psa = psapool.tile([_QROWS, ha], f32, tag="psa")
                psb = psbpool.tile([_QROWS, ha], f32, tag="psb")
                for m in range(0, ha, 512):
                    nc.tensor.matmul(psa[:, m:m + 512], wt[:],
                                     vin[:, off + m:off + m + 512],
                                     start=True, stop=True)
                    nc.tensor.matmul(psb[:, m:m + 512], wt[:],
                                     vin[:, off + ha + m:off + ha + m + 512],
                                     start=True, stop=True)
                nc.scalar.copy(out=yat[:, h:h + ha], in_=psa[:])
                nc.vector.tensor_copy(ybt[:, h:h + ha], psb[:])
                h += ha

                # paced late input issues (chunk k+3 after drain of chunk k)
                ki = k + 3
                if 5 <= ki < len(_CHUNKS):
                    nc.scalar.dma_start(
                        out=vin[:, offs[ki]:offs[ki + 1]],
                        in_=v[:, offs[ki]:offs[ki + 1]])

                # out-DMA per chunk pair per side: A-halves (ACT-drained) on
                # the Sync HWDGE ring after all its input issues, B-halves
                # on Scalar. (gpsimd SWDGE adds ~3us first-byte latency.)
                if k % 2 == 1 or k == len(_CHUNKS) - 1:
                    nc.sync.dma_start(out=ya_d[:, hmark:h],
                                      in_=yat[:, hmark:h])
                    nc.scalar.dma_start(out=yb_d[:, hmark:h],
                                        in_=ybt[:, hmark:h])
                    hmark = h

    nc.compile()
    return nc


def _get_nc():
    if "nc" not in _cache:
        _cache["nc"] = _build()
    return _cache["nc"]


def _haar_structure_ok(A):
    """A must be the 2-tap stride-2 filter bank with taps (h, h) lowpass /
    (-h, h) highpass, which is what the butterflies hardcode."""
    if A.shape != (_N, _N):
        return False
    h = A[0, 0]
    if not np.isfinite(h) or abs(h) < 1e-8:
        return False
    expect = np.zeros((_N, _N), dtype=np.float32)
    for i in range(_N // 2):
        expect[i, 2 * i] = h
        expect[i, 2 * i + 1] = h
        expect[_N // 2 + i, 2 * i] = -h
        expect[_N // 2 + i, 2 * i + 1] = h
    return bool(np.allclose(A, expect, rtol=1e-5, atol=1e-7))


def _reference_host(x, A):
    """Generic numpy fallback (slow) for non-Haar A."""
    y = np.einsum("ai,nijkc->najkc", A, x, optimize=True)
    y = np.einsum("bj,najkc->nabkc", A, y, optimize=True)
    y = np.einsum("dk,nabkc->nabdc", A, y, optimize=True)
    return np.moveaxis(y, -1, 1)


def _assemble(y_full, B, C):
    """Slice transformed volumes y_full [B, C, 128,128,128] into the
    reference's subband concat [B, 64, 64, 64, 8*C] (incl. the duplicated
    HHH octant the reference produces)."""
    L, H = slice(0, 64), slice(64, 128)
    bands = [(L, L, L), (H, L, L), (L, H, L), (H, H, L),
             (L, L, H), (H, H, H), (L, H, H), (H, H, H)]
    out = np.empty((B, 64, 64, 64, 8 * C), dtype=np.float32)
    for s, (sa, sb, sd) in enumerate(bands):
        out[..., s * C:(s + 1) * C] = np.moveaxis(y_full[:, :, sa, sb, sd], 1, -1)
    return out


def kernel(x, A):
    from concourse.bass_utils import run_bass_kernel_spmd

    x = np.asarray(x, dtype=np.float32)
    A = np.asarray(A, dtype=np.float32)
    B, _, _, _, C = x.shape
    assert (B, C) == (2, 4) and x.shape[1:4] == (_N, _N, _N)

    if not _haar_structure_ok(A):
        return _assemble(_reference_host(x, A), B, C)

    h = float(A[0, 0])
    # W maps partition (ei, ej, ek, kh) -> (octant o, kh): all three
    # butterfly passes at once, h^3 folded in. lhsT = W.T [128, 112].
    sgn = np.array([[1.0, 1.0], [-1.0, 1.0]], dtype=np.float32)
    Wm = np.zeros((_QROWS, _N), dtype=np.float32)
    h3 = np.float32(h ** 3)
    for o, (lhi, lhj, lhk) in enumerate(_OCT):
        for ei in range(2):
            for ej in range(2):
                for ek in range(2):
                    c = h3 * sgn[lhi, ei] * sgn[lhj, ej] * sgn[lhk, ek]
                    for kh in range(16):
                        Wm[o * 16 + kh,
                           ei * 64 + ej * 32 + ek * 16 + kh] = c
    wT = np.ascontiguousarray(Wm.T.astype(np.float16))

    # Host layout: p = (i&1, j&1, k&1, (k>>1)&15), f = (i>>1, j>>1, k>>5)
    xs = np.transpose(x, (0, 4, 1, 2, 3))               # [B, C, i, j, k]
    t = xs.reshape(_CORES, 64, 2, 64, 2, 4, 16, 2)      # g,mi,ei,mj,ej,ml,kh,ek
    t = t.transpose(0, 2, 4, 7, 6, 1, 3, 5)             # g,ei,ej,ek,kh,mi,mj,ml
    V = np.ascontiguousarray(t).astype(np.float16).reshape(_CORES, _N, _F)

    in_maps = [{"v": V[g], "w": wT} for g in range(_CORES)]
    nc = _get_nc()
    res = run_bass_kernel_spmd(nc, in_maps, list(range(_CORES)))

    # Device rows (o, kh), cols (mi, mj, ml). Within-octant output index:
    # a = 64*lhi + mi, b = 64*lhj + mj, d = 64*lhk + 16*ml + kh.
    out = np.empty((B, 64, 64, 64, 8 * C), dtype=np.float32)
    for g in range(_CORES):
        b, c = g // C, g % C
        # re-interleave the ACT/DVE half-chunk regions into original cols
        ydev = np.empty((_QROWS, _F), dtype=np.float16)
        yag = np.asarray(res.results[g]["ya"])
        ybg = np.asarray(res.results[g]["yb"])
        off = hh = 0
        for cw in _CHUNKS:
            ha = cw // 2
            ydev[:, off:off + ha] = yag[:, hh:hh + ha]
            ydev[:, off + ha:off + cw] = ybg[:, hh:hh + ha]
            off += cw
            hh += ha
        z = ydev.reshape(7, 16, 64, 64, 4)
        z = z.transpose(0, 2, 3, 4, 1).astype(np.float32)  # o,mi,mj,ml,kh
        z = z.reshape(7, 64, 64, 64)
        for s in range(8):
            out[b, :, :, :, s * C + c] = z[_OMAP[s]]
    return out


# revision 22
# speedup vs baseline: 1.0391x; 1.0391x over previous
"""Trainium2 Bass kernel for a separable 3D Haar DWT (nn_DWT3D).

Problem: x [2, 128, 128, 128, 4] fp32, A [128, 128] (orthonormal Haar
analysis filter bank, 2-tap stride-2). Output: subband concat
[2, 64, 64, 64, 32].

Strategy (8 NeuronCores):
- Data-parallel over (batch, channel): B*C = 8 independent [128,128,128]
  volume transforms, one per core. Host deinterleaves channels on the
  way in and assembles the subband concat on the way out (numpy).
- KEY TRICK: the host lays each volume out with partition index
  p = (i&1, j&1, k&1, (k>>1)&15) and free index f = (i>>1, j>>1, k>>5).
  ALL THREE Haar butterfly passes then act on the partition axis, so ONE
  128-wide PE matmul (butterfly_i (x) butterfly_j (x) butterfly_k (x)
  I_16, 8 nonzeros per row, h^3 folded in) performs the whole 3D
  transform. No vector-engine pass at all.
- fp16 everywhere off-chip (tolerance is 2e-2; fp16 keeps us ~1e-3):
  halves DMA bytes and runs the PE at 1 cycle/row instead of fp32's
  effective 8 (2 half-speed passes).
- The reference's subband concat never emits the (a=H, b=L, d=H) octant
  (its HLH slice equals HHH), so only 112 of 128 output rows are
  computed, drained, and DMA'd out.
- Per 2048-col chunk: SP issues in-DMA -> 4 PE matmuls (512 cols each,
  one PSUM bank) -> drain PSUM->SBUF fp16 split across ACT and DVE ->
  GpSimd (SWDGE) issues out-DMA.
"""

import numpy as np

_N = 128
_CORES = 8
_F = _N * _N  # 16384 free columns per volume
# chunk widths; middle chunks give 4KB DMA row-descriptors, small edges
# shorten pipeline fill and drain. DMA count must stay low (~20): the DMA
# semaphore pool is small and recycles in issue order.
_CHUNKS = [1024] + [2048] * 7 + [1024]
_QROWS = 112  # 7 of 8 output octants * 16 kh rows

# Octants (lhi, lhj, lhk) in device output row order; (1, 0, 1) omitted.
_OCT = [(0, 0, 0), (1, 0, 0), (0, 1, 0), (1, 1, 0),
        (0, 0, 1), (0, 1, 1), (1, 1, 1)]
# reference band order LLL,LLH,LHL,LHH,HLL,HLH,HHL,HHH -> index into _OCT
_OMAP = [0, 1, 2, 3, 4, 6, 5, 6]

_cache = {}


def _build():
    import concourse.mybir as mybir
    from concourse import bacc
    from concourse.tile import TileContext

    nc = bacc.Bacc("TRN2", target_bir_lowering=False, debug=False,
                   num_devices=_CORES)
    f16 = mybir.dt.float16
    f32 = mybir.dt.float32
    v = nc.dram_tensor("v", [_N, _F], f16, kind="ExternalInput")
    w = nc.dram_tensor("w", [_N, _QROWS], f16, kind="ExternalInput")
    # two output regions: ACT-drained (even 512-col banks) and DVE-drained
    # (odd banks); host re-interleaves
    ya_d = nc.dram_tensor("ya", [_QROWS, _F // 2], f16, kind="ExternalOutput")
    yb_d = nc.dram_tensor("yb", [_QROWS, _F // 2], f16, kind="ExternalOutput")

    with TileContext(nc) as tc:
        with (
            tc.tile_pool(name="wpool", bufs=1) as wpool,
            tc.tile_pool(name="vin", bufs=4) as vpool,
            tc.tile_pool(name="ya", bufs=1) as yapool,
            tc.tile_pool(name="yb", bufs=1) as ybpool,
            tc.tile_pool(name="psa", bufs=2, space="PSUM") as psapool,
            tc.tile_pool(name="psb", bufs=2, space="PSUM") as psbpool,
        ):
            wt = wpool.tile([_N, _QROWS], f16)
            nc.scalar.dma_start(out=wt[:], in_=w[:])

            yat = yapool.tile([_QROWS, _F // 2], f16)
            ybt = ybpool.tile([_QROWS, _F // 2], f16)

            offs = np.cumsum([0] + _CHUNKS)
            h = 0     # per-side output col offset
            hmark = 0  # start of the not-yet-DMA'd output region
            for k, cw in enumerate(_CHUNKS):
                off = offs[k]
                ha = cw // 2

                # per-chunk input tile from a limited pool: input DMA k
                # acquires the buffer chunk k-4's matmuls used, which paces
                # the input descriptor stream so it cannot run arbitrarily
                # far ahead of the output stream (DMA engines service
                # descriptors roughly FIFO, so a deep input backlog would
                # delay output start)
                vt = vpool.tile([_N, cw], f16, tag="vin")
                nc.sync.dma_start(out=vt[:], in_=v[:, off:off + cw])

                # independent PSUM tiles for the ACT- and DVE-drained halves
                psa = psapool.tile([_QROWS, ha], f32, tag="psa")
                psb = psbpool.tile([_QROWS, ha], f32, tag="psb")
                for m in range(0, ha, 512):
                    nc.tensor.matmul(psa[:, m:m + 512], wt[:],
                                     vt[:, m:m + 512],
                                     start=True, stop=True)
                    nc.tensor.matmul(psb[:, m:m + 512], wt[:],
                                     vt[:, ha + m:ha + m + 512],
                                     start=True, stop=True)
                nc.scalar.copy(out=yat[:, h:h + ha], in_=psa[:])
                nc.vector.tensor_copy(ybt[:, h:h + ha], psb[:])
                h += ha

                # out-DMA per chunk pair per side, both on the Scalar HWDGE
                # ring (gpsimd SWDGE adds ~3us first-byte latency)
                if k % 2 == 1 or k == len(_CHUNKS) - 1:
                    nc.scalar.dma_start(out=ya_d[:, hmark:h],
                                        in_=yat[:, hmark:h])
                    nc.scalar.dma_start(out=yb_d[:, hmark:h],
                                        in_=ybt[:, hmark:h])
                    hmark = h

    nc.compile()
    return nc


def _get_nc():
    if "nc" not in _cache:
        _cache["nc"] = _build()
    return _cache["nc"]


def _haar_structure_ok(A):
    """A must be the 2-tap stride-2 filter bank with taps (h, h) lowpass /
    (-h, h) highpass, which is what the butterflies hardcode."""
    if A.shape != (_N, _N):
        return False
    h = A[0, 0]
    if not np.isfinite(h) or abs(h) < 1e-8:
        return False
    expect = np.zeros((_N, _N), dtype=np.float32)
    for i in range(_N // 2):
        expect[i, 2 * i] = h
        expect[i, 2 * i + 1] = h
        expect[_N // 2 + i, 2 * i] = -h
        expect[_N // 2 + i, 2 * i + 1] = h
    return bool(np.allclose(A, expect, rtol=1e-5, atol=1e-7))


def _reference_host(x, A):
    """Generic numpy fallback (slow) for non-Haar A."""
    y = np.einsum("ai,nijkc->najkc", A, x, optimize=True)
    y = np.einsum("bj,najkc->nabkc", A, y, optimize=True)
    y = np.einsum("dk,nabkc->nabdc", A, y, optimize=True)
    return np.moveaxis(y, -1, 1)


def _assemble(y_full, B, C):
    """Slice transformed volumes y_full [B, C, 128,128,128] into the
    reference's subband concat [B, 64, 64, 64, 8*C] (incl. the duplicated
    HHH octant the reference produces)."""
    L, H = slice(0, 64), slice(64, 128)
    bands = [(L, L, L), (H, L, L), (L, H, L), (H, H, L),
             (L, L, H), (H, H, H), (L, H, H), (H, H, H)]
    out = np.empty((B, 64, 64, 64, 8 * C), dtype=np.float32)
    for s, (sa, sb, sd) in enumerate(bands):
        out[..., s * C:(s + 1) * C] = np.moveaxis(y_full[:, :, sa, sb, sd], 1, -1)
    return out


def kernel(x, A):
    from concourse.bass_utils import run_bass_kernel_spmd

    x = np.asarray(x, dtype=np.float32)
    A = np.asarray(A, dtype=np.float32)
    B, _, _, _, C = x.shape
    assert (B, C) == (2, 4) and x.shape[1:4] == (_N, _N, _N)

    if not _haar_structure_ok(A):
        return _assemble(_reference_host(x, A), B, C)

    h = float(A[0, 0])
    # W maps partition (ei, ej, ek, kh) -> (octant o, kh): all three
    # butterfly passes at once, h^3 folded in. lhsT = W.T [128, 112].
    sgn = np.array([[1.0, 1.0], [-1.0, 1.0]], dtype=np.float32)
    Wm = np.zeros((_QROWS, _N), dtype=np.float32)
    h3 = np.float32(h ** 3)
    for o, (lhi, lhj, lhk) in enumerate(_OCT):
        for ei in range(2):
            for ej in range(2):
                for ek in range(2):
                    c = h3 * sgn[lhi, ei] * sgn[lhj, ej] * sgn[lhk, ek]
                    for kh in range(16):
                        Wm[o * 16 + kh,
                           ei * 64 + ej * 32 + ek * 16 + kh] = c
    wT = np.ascontiguousarray(Wm.T.astype(np.float16))

    # Host layout: p = (i&1, j&1, k&1, (k>>1)&15), f = (i>>1, j>>1, k>>5)
    xs = np.transpose(x, (0, 4, 1, 2, 3))               # [B, C, i, j, k]
    t = xs.reshape(_CORES, 64, 2, 64, 2, 4, 16, 2)      # g,mi,ei,mj,ej,ml,kh,ek
    t = t.transpose(0, 2, 4, 7, 6, 1, 3, 5)             # g,ei,ej,ek,kh,mi,mj,ml
    V = np.ascontiguousarray(t).astype(np.float16).reshape(_CORES, _N, _F)

    in_maps = [{"v": V[g], "w": wT} for g in range(_CORES)]
    nc = _get_nc()
    res = run_bass_kernel_spmd(nc, in_maps, list(range(_CORES)))

    # Device rows (o, kh), cols (mi, mj, ml). Within-octant output index:
    # a = 64*lhi + mi, b = 64*lhj + mj, d = 64*lhk + 16*ml + kh.
    out = np.empty((B, 64, 64, 64, 8 * C), dtype=np.float32)
    for g in range(_CORES):
        b, c = g // C, g % C
        # re-interleave the ACT/DVE half-chunk regions into original cols
        ydev = np.empty((_QROWS, _F), dtype=np.float16)
        yag = np.asarray(res.results[g]["ya"])
        ybg = np.asarray(res.results[g]["yb"])
        off = hh = 0
        for cw in _CHUNKS:
            ha = cw // 2
            ydev[:, off:off + ha] = yag[:, hh:hh + ha]
            ydev[:, off + ha:off + cw] = ybg[:, hh:hh + ha]
            off += cw
            hh += ha
        z = ydev.reshape(7, 16, 64, 64, 4)
        z = z.transpose(0, 2, 3, 4, 1).astype(np.float32)  # o,mi,mj,ml,kh
        z = z.reshape(7, 64, 64, 64)
        for s in range(8):
            out[b, :, :, :, s * C + c] = z[_OMAP[s]]
    return out


# revision 26
# speedup vs baseline: 1.1285x; 1.0860x over previous
"""Trainium2 Bass kernel for a separable 3D Haar DWT (nn_DWT3D).

Problem: x [2, 128, 128, 128, 4] fp32, A [128, 128] (orthonormal Haar
analysis filter bank, 2-tap stride-2). Output: subband concat
[2, 64, 64, 64, 32].

Strategy (8 NeuronCores):
- Data-parallel over (batch, channel): B*C = 8 independent [128,128,128]
  volume transforms, one per core. Host deinterleaves channels on the
  way in and assembles the subband concat on the way out (numpy).
- KEY TRICK: the host lays each volume out with partition index
  p = (i&1, j&1, k&1, (k>>1)&15) and free index f = (i>>1, j>>1, k>>5).
  ALL THREE Haar butterfly passes then act on the partition axis, so ONE
  128-wide PE matmul (butterfly_i (x) butterfly_j (x) butterfly_k (x)
  I_16, 8 nonzeros per row, h^3 folded in) performs the whole 3D
  transform. No vector-engine pass at all.
- fp16 everywhere off-chip (tolerance is 2e-2; fp16 keeps us ~1e-3):
  halves DMA bytes and runs the PE at 1 cycle/row instead of fp32's
  effective 8 (2 half-speed passes).
- The reference's subband concat never emits the (a=H, b=L, d=H) octant
  (its HLH slice equals HHH), so only 112 of 128 output rows are
  computed, drained, and DMA'd out.
- Per 2048-col chunk: SP issues in-DMA -> 4 PE matmuls (512 cols each,
  one PSUM bank) -> drain PSUM->SBUF fp16 split across ACT and DVE ->
  GpSimd (SWDGE) issues out-DMA.
"""

import numpy as np

_N = 128
_CORES = 8
_F = _N * _N  # 16384 free columns per volume
# input DMA widths (decoupled from compute chunks): few, fat transfers
# (4-8KB row descriptors) keep the queue deep; DMA count must stay low
# (~19): the DMA semaphore pool is small and recycles in issue order.
_INW = [2048, 4096, 4096, 4096, 2048]
# compute chunk width: per chunk one 512-col matmul into each of two
# 1-bank PSUM tiles (ACT-drained A half, DVE-drained B half)
_CW = 1024
_NCH = _F // _CW
# per-side output col counts per out-DMA group (accumulates 512/chunk);
# small final groups shorten the drain tail
_OUTG = [2048, 2048, 2048, 1024, 1024]
_QROWS = 112  # 7 of 8 output octants * 16 kh rows

# Octants (lhi, lhj, lhk) in device output row order; (1, 0, 1) omitted.
_OCT = [(0, 0, 0), (1, 0, 0), (0, 1, 0), (1, 1, 0),
        (0, 0, 1), (0, 1, 1), (1, 1, 1)]
# reference band order LLL,LLH,LHL,LHH,HLL,HLH,HHL,HHH -> index into _OCT
_OMAP = [0, 1, 2, 3, 4, 6, 5, 6]

_cache = {}


def _build():
    import concourse.mybir as mybir
    from concourse import bacc
    from concourse.tile import TileContext

    nc = bacc.Bacc("TRN2", target_bir_lowering=False, debug=False,
                   num_devices=_CORES)
    f16 = mybir.dt.float16
    f32 = mybir.dt.float32
    v = nc.dram_tensor("v", [_N, _F], f16, kind="ExternalInput")
    w = nc.dram_tensor("w", [_N, _QROWS], f16, kind="ExternalInput")
    # two output regions: ACT-drained (even 512-col banks) and DVE-drained
    # (odd banks); host re-interleaves
    ya_d = nc.dram_tensor("ya", [_QROWS, _F // 2], f16, kind="ExternalOutput")
    yb_d = nc.dram_tensor("yb", [_QROWS, _F // 2], f16, kind="ExternalOutput")

    with TileContext(nc) as tc:
        with (
            tc.tile_pool(name="wpool", bufs=1) as wpool,
            tc.tile_pool(name="vin", bufs=1) as vpool,
            tc.tile_pool(name="ya", bufs=1) as yapool,
            tc.tile_pool(name="yb", bufs=1) as ybpool,
            tc.tile_pool(name="psa", bufs=4, space="PSUM") as psapool,
            tc.tile_pool(name="psb", bufs=4, space="PSUM") as psbpool,
        ):
            wt = wpool.tile([_N, _QROWS], f16)
            nc.scalar.dma_start(out=wt[:], in_=w[:])

            vin = vpool.tile([_N, _F], f16)
            yat = yapool.tile([_QROWS, _F // 2], f16)
            ybt = ybpool.tile([_QROWS, _F // 2], f16)

            ioff = 0
            for iw in _INW:
                nc.sync.dma_start(out=vin[:, ioff:ioff + iw],
                                  in_=v[:, ioff:ioff + iw])
                ioff += iw

            gends = np.cumsum(_OUTG)
            gi = 0
            hmark = 0  # start of the not-yet-DMA'd output region
            for k in range(_NCH):
                off = k * _CW
                h = k * 512
                ha = _CW // 2

                # one 1-bank PSUM tile per matmul per side; 4 bufs each so
                # the PE stays ahead of the drains (p-state ramp)
                psa = psapool.tile([_QROWS, 512], f32, tag="psa")
                psb = psbpool.tile([_QROWS, 512], f32, tag="psb")
                nc.tensor.matmul(psa[:], wt[:], vin[:, off:off + 512],
                                 start=True, stop=True)
                nc.tensor.matmul(psb[:], wt[:], vin[:, off + ha:off + ha + 512],
                                 start=True, stop=True)
                nc.scalar.copy(out=yat[:, h:h + 512], in_=psa[:])
                nc.vector.tensor_copy(ybt[:, h:h + 512], psb[:])

                # grouped out-DMAs: A-halves on the Sync HWDGE ring (idle
                # after the input issues), B-halves on Scalar (gpsimd SWDGE
                # adds ~3us first-byte latency)
                if gi < len(gends) and (h + 512) == gends[gi]:
                    he = gends[gi]
                    nc.sync.dma_start(out=ya_d[:, hmark:he],
                                      in_=yat[:, hmark:he])
                    nc.scalar.dma_start(out=yb_d[:, hmark:he],
                                        in_=ybt[:, hmark:he])
                    hmark = he
                    gi += 1

    nc.compile()
    return nc


def _get_nc():
    if "nc" not in _cache:
        _cache["nc"] = _build()
    return _cache["nc"]


def _haar_structure_ok(A):
    """A must be the 2-tap stride-2 filter bank with taps (h, h) lowpass /
    (-h, h) highpass, which is what the butterflies hardcode."""
    if A.shape != (_N, _N):
        return False
    h = A[0, 0]
    if not np.isfinite(h) or abs(h) < 1e-8:
        return False
    expect = np.zeros((_N, _N), dtype=np.float32)
    for i in range(_N // 2):
        expect[i, 2 * i] = h
        expect[i, 2 * i + 1] = h
        expect[_N // 2 + i, 2 * i] = -h
        expect[_N // 2 + i, 2 * i + 1] = h
    return bool(np.allclose(A, expect, rtol=1e-5, atol=1e-7))


def _reference_host(x, A):
    """Generic numpy fallback (slow) for non-Haar A."""
    y = np.einsum("ai,nijkc->najkc", A, x, optimize=True)
    y = np.einsum("bj,najkc->nabkc", A, y, optimize=True)
    y = np.einsum("dk,nabkc->nabdc", A, y, optimize=True)
    return np.moveaxis(y, -1, 1)


def _assemble(y_full, B, C):
    """Slice transformed volumes y_full [B, C, 128,128,128] into the
    reference's subband concat [B, 64, 64, 64, 8*C] (incl. the duplicated
    HHH octant the reference produces)."""
    L, H = slice(0, 64), slice(64, 128)
    bands = [(L, L, L), (H, L, L), (L, H, L), (H, H, L),
             (L, L, H), (H, H, H), (L, H, H), (H, H, H)]
    out = np.empty((B, 64, 64, 64, 8 * C), dtype=np.float32)
    for s, (sa, sb, sd) in enumerate(bands):
        out[..., s * C:(s + 1) * C] = np.moveaxis(y_full[:, :, sa, sb, sd], 1, -1)
    return out


def kernel(x, A):
    from concourse.bass_utils import run_bass_kernel_spmd

    x = np.asarray(x, dtype=np.float32)
    A = np.asarray(A, dtype=np.float32)
    B, _, _, _, C = x.shape
    assert (B, C) == (2, 4) and x.shape[1:4] == (_N, _N, _N)

    if not _haar_structure_ok(A):
        return _assemble(_reference_host(x, A), B, C)

    h = float(A[0, 0])
    # W maps partition (ei, ej, ek, kh) -> (octant o, kh): all three
    # butterfly passes at once, h^3 folded in. lhsT = W.T [128, 112].
    sgn = np.array([[1.0, 1.0], [-1.0, 1.0]], dtype=np.float32)
    Wm = np.zeros((_QROWS, _N), dtype=np.float32)
    h3 = np.float32(h ** 3)
    for o, (lhi, lhj, lhk) in enumerate(_OCT):
        for ei in range(2):
            for ej in range(2):
                for ek in range(2):
                    c = h3 * sgn[lhi, ei] * sgn[lhj, ej] * sgn[lhk, ek]
                    for kh in range(16):
                        Wm[o * 16 + kh,
                           ei * 64 + ej * 32 + ek * 16 + kh] = c
    wT = np.ascontiguousarray(Wm.T.astype(np.float16))

    # Host layout: p = (i&1, j&1, k&1, (k>>1)&15), f = (i>>1, j>>1, k>>5)
    xs = np.transpose(x, (0, 4, 1, 2, 3))               # [B, C, i, j, k]
    t = xs.reshape(_CORES, 64, 2, 64, 2, 4, 16, 2)      # g,mi,ei,mj,ej,ml,kh,ek
    t = t.transpose(0, 2, 4, 7, 6, 1, 3, 5)             # g,ei,ej,ek,kh,mi,mj,ml
    V = np.ascontiguousarray(t).astype(np.float16).reshape(_CORES, _N, _F)

    in_maps = [{"v": V[g], "w": wT} for g in range(_CORES)]
    nc = _get_nc()
    res = run_bass_kernel_spmd(nc, in_maps, list(range(_CORES)))

    # Device rows (o, kh), cols (mi, mj, ml). Within-octant output index:
    # a = 64*lhi + mi, b = 64*lhj + mj, d = 64*lhk + 16*ml + kh.
    out = np.empty((B, 64, 64, 64, 8 * C), dtype=np.float32)
    for g in range(_CORES):
        b, c = g // C, g % C
        # re-interleave the ACT/DVE half-chunk regions into original cols
        ydev = np.empty((_QROWS, _NCH, 2, 512), dtype=np.float16)
        ydev[:, :, 0, :] = np.asarray(
            res.results[g]["ya"]).reshape(_QROWS, _NCH, 512)
        ydev[:, :, 1, :] = np.asarray(
            res.results[g]["yb"]).reshape(_QROWS, _NCH, 512)
        z = ydev.reshape(7, 16, 64, 64, 4)
        z = z.transpose(0, 2, 3, 4, 1).astype(np.float32)  # o,mi,mj,ml,kh
        z = z.reshape(7, 64, 64, 64)
        for s in range(8):
            out[b, :, :, :, s * C + c] = z[_OMAP[s]]
    return out


# revision 28
# speedup vs baseline: 1.1646x; 1.0320x over previous
"""Trainium2 Bass kernel for a separable 3D Haar DWT (nn_DWT3D).

Problem: x [2, 128, 128, 128, 4] fp32, A [128, 128] (orthonormal Haar
analysis filter bank, 2-tap stride-2). Output: subband concat
[2, 64, 64, 64, 32].

Strategy (8 NeuronCores):
- Data-parallel over (batch, channel): B*C = 8 independent [128,128,128]
  volume transforms, one per core. Host deinterleaves channels on the
  way in and assembles the subband concat on the way out (numpy).
- KEY TRICK: the host lays each volume out with partition index
  p = (i&1, j&1, k&1, (k>>1)&15) and free index f = (i>>1, j>>1, k>>5).
  ALL THREE Haar butterfly passes then act on the partition axis, so ONE
  128-wide PE matmul (butterfly_i (x) butterfly_j (x) butterfly_k (x)
  I_16, 8 nonzeros per row, h^3 folded in) performs the whole 3D
  transform. No vector-engine pass at all.
- fp16 everywhere off-chip (tolerance is 2e-2; fp16 keeps us ~1e-3):
  halves DMA bytes and runs the PE at 1 cycle/row instead of fp32's
  effective 8 (2 half-speed passes).
- The reference's subband concat never emits the (a=H, b=L, d=H) octant
  (its HLH slice equals HHH), so only 112 of 128 output rows are
  computed, drained, and DMA'd out.
- Per 2048-col chunk: SP issues in-DMA -> 4 PE matmuls (512 cols each,
  one PSUM bank) -> drain PSUM->SBUF fp16 split across ACT and DVE ->
  GpSimd (SWDGE) issues out-DMA.
"""

import numpy as np

_N = 128
_CORES = 8
_F = _N * _N  # 16384 free columns per volume
# input DMA widths (decoupled from compute chunks): few, fat transfers
# (4-8KB row descriptors) keep the queue deep; DMA count must stay low
# (~19): the DMA semaphore pool is small and recycles in issue order.
_INW = [1024, 4096, 4096, 4096, 2048, 1024]
# compute chunk width: per chunk one 512-col matmul into each of two
# 1-bank PSUM tiles (ACT-drained A half, DVE-drained B half)
_CW = 1024
_NCH = _F // _CW
# per-side output col counts per out-DMA group (accumulates 512/chunk);
# small final groups shorten the drain tail
_OUTG = [2048, 2048, 2048, 1024, 1024]
_QROWS = 112  # 7 of 8 output octants * 16 kh rows

# Octants (lhi, lhj, lhk) in device output row order; (1, 0, 1) omitted.
_OCT = [(0, 0, 0), (1, 0, 0), (0, 1, 0), (1, 1, 0),
        (0, 0, 1), (0, 1, 1), (1, 1, 1)]
# reference band order LLL,LLH,LHL,LHH,HLL,HLH,HHL,HHH -> index into _OCT
_OMAP = [0, 1, 2, 3, 4, 6, 5, 6]

_cache = {}


def _build():
    import concourse.mybir as mybir
    from concourse import bacc
    from concourse.tile import TileContext

    nc = bacc.Bacc("TRN2", target_bir_lowering=False, debug=False,
                   num_devices=_CORES)
    f16 = mybir.dt.float16
    f32 = mybir.dt.float32
    v = nc.dram_tensor("v", [_N, _F], f16, kind="ExternalInput")
    w = nc.dram_tensor("w", [_N, _QROWS], f16, kind="ExternalInput")
    # two output regions: ACT-drained (even 512-col banks) and DVE-drained
    # (odd banks); host re-interleaves
    ya_d = nc.dram_tensor("ya", [_QROWS, _F // 2], f16, kind="ExternalOutput")
    yb_d = nc.dram_tensor("yb", [_QROWS, _F // 2], f16, kind="ExternalOutput")

    with TileContext(nc) as tc:
        with (
            tc.tile_pool(name="wpool", bufs=1) as wpool,
            tc.tile_pool(name="vin", bufs=1) as vpool,
            tc.tile_pool(name="ya", bufs=1) as yapool,
            tc.tile_pool(name="yb", bufs=1) as ybpool,
            tc.tile_pool(name="psa", bufs=4, space="PSUM") as psapool,
            tc.tile_pool(name="psb", bufs=4, space="PSUM") as psbpool,
        ):
            wt = wpool.tile([_N, _QROWS], f16)
            nc.scalar.dma_start(out=wt[:], in_=w[:])

            vin = vpool.tile([_N, _F], f16)
            yat = yapool.tile([_QROWS, _F // 2], f16)
            ybt = ybpool.tile([_QROWS, _F // 2], f16)

            ioff = 0
            for iw in _INW:
                nc.sync.dma_start(out=vin[:, ioff:ioff + iw],
                                  in_=v[:, ioff:ioff + iw])
                ioff += iw

            gends = np.cumsum(_OUTG)
            gi = 0
            hmark = 0  # start of the not-yet-DMA'd output region
            for k in range(_NCH):
                off = k * _CW
                h = k * 512
                ha = _CW // 2

                # one 1-bank PSUM tile per matmul per side; 4 bufs each so
                # the PE stays ahead of the drains (p-state ramp)
                psa = psapool.tile([_QROWS, 512], f32, tag="psa")
                psb = psbpool.tile([_QROWS, 512], f32, tag="psb")
                nc.tensor.matmul(psa[:], wt[:], vin[:, off:off + 512],
                                 start=True, stop=True)
                nc.tensor.matmul(psb[:], wt[:], vin[:, off + ha:off + ha + 512],
                                 start=True, stop=True)
                nc.scalar.copy(out=yat[:, h:h + 512], in_=psa[:])
                nc.vector.tensor_copy(ybt[:, h:h + 512], psb[:])

                # grouped out-DMAs, all from the otherwise-idle GpSimd
                # (issuing them on Sync queues them behind ALL input in the
                # same FIFO ring; on Scalar they crowd out the drains)
                if gi < len(gends) and (h + 512) == gends[gi]:
                    he = gends[gi]
                    nc.gpsimd.dma_start(out=ya_d[:, hmark:he],
                                        in_=yat[:, hmark:he])
                    nc.gpsimd.dma_start(out=yb_d[:, hmark:he],
                                        in_=ybt[:, hmark:he])
                    hmark = he
                    gi += 1

    nc.compile()
    return nc


def _get_nc():
    if "nc" not in _cache:
        _cache["nc"] = _build()
    return _cache["nc"]


def _haar_structure_ok(A):
    """A must be the 2-tap stride-2 filter bank with taps (h, h) lowpass /
    (-h, h) highpass, which is what the butterflies hardcode."""
    if A.shape != (_N, _N):
        return False
    h = A[0, 0]
    if not np.isfinite(h) or abs(h) < 1e-8:
        return False
    expect = np.zeros((_N, _N), dtype=np.float32)
    for i in range(_N // 2):
        expect[i, 2 * i] = h
        expect[i, 2 * i + 1] = h
        expect[_N // 2 + i, 2 * i] = -h
        expect[_N // 2 + i, 2 * i + 1] = h
    return bool(np.allclose(A, expect, rtol=1e-5, atol=1e-7))


def _reference_host(x, A):
    """Generic numpy fallback (slow) for non-Haar A."""
    y = np.einsum("ai,nijkc->najkc", A, x, optimize=True)
    y = np.einsum("bj,najkc->nabkc", A, y, optimize=True)
    y = np.einsum("dk,nabkc->nabdc", A, y, optimize=True)
    return np.moveaxis(y, -1, 1)


def _assemble(y_full, B, C):
    """Slice transformed volumes y_full [B, C, 128,128,128] into the
    reference's subband concat [B, 64, 64, 64, 8*C] (incl. the duplicated
    HHH octant the reference produces)."""
    L, H = slice(0, 64), slice(64, 128)
    bands = [(L, L, L), (H, L, L), (L, H, L), (H, H, L),
             (L, L, H), (H, H, H), (L, H, H), (H, H, H)]
    out = np.empty((B, 64, 64, 64, 8 * C), dtype=np.float32)
    for s, (sa, sb, sd) in enumerate(bands):
        out[..., s * C:(s + 1) * C] = np.moveaxis(y_full[:, :, sa, sb, sd], 1, -1)
    return out


def kernel(x, A):
    from concourse.bass_utils import run_bass_kernel_spmd

    x = np.asarray(x, dtype=np.float32)
    A = np.asarray(A, dtype=np.float32)
    B, _, _, _, C = x.shape
    assert (B, C) == (2, 4) and x.shape[1:4] == (_N, _N, _N)

    if not _haar_structure_ok(A):
        return _assemble(_reference_host(x, A), B, C)

    h = float(A[0, 0])
    # W maps partition (ei, ej, ek, kh) -> (octant o, kh): all three
    # butterfly passes at once, h^3 folded in. lhsT = W.T [128, 112].
    sgn = np.array([[1.0, 1.0], [-1.0, 1.0]], dtype=np.float32)
    Wm = np.zeros((_QROWS, _N), dtype=np.float32)
    h3 = np.float32(h ** 3)
    for o, (lhi, lhj, lhk) in enumerate(_OCT):
        for ei in range(2):
            for ej in range(2):
                for ek in range(2):
                    c = h3 * sgn[lhi, ei] * sgn[lhj, ej] * sgn[lhk, ek]
                    for kh in range(16):
                        Wm[o * 16 + kh,
                           ei * 64 + ej * 32 + ek * 16 + kh] = c
    wT = np.ascontiguousarray(Wm.T.astype(np.float16))

    # Host layout: p = (i&1, j&1, k&1, (k>>1)&15), f = (i>>1, j>>1, k>>5)
    xs = np.transpose(x, (0, 4, 1, 2, 3))               # [B, C, i, j, k]
    t = xs.reshape(_CORES, 64, 2, 64, 2, 4, 16, 2)      # g,mi,ei,mj,ej,ml,kh,ek
    t = t.transpose(0, 2, 4, 7, 6, 1, 3, 5)             # g,ei,ej,ek,kh,mi,mj,ml
    V = np.ascontiguousarray(t).astype(np.float16).reshape(_CORES, _N, _F)

    in_maps = [{"v": V[g], "w": wT} for g in range(_CORES)]
    nc = _get_nc()
    res = run_bass_kernel_spmd(nc, in_maps, list(range(_CORES)))

    # Device rows (o, kh), cols (mi, mj, ml). Within-octant output index:
    # a = 64*lhi + mi, b = 64*lhj + mj, d = 64*lhk + 16*ml + kh.
    out = np.empty((B, 64, 64, 64, 8 * C), dtype=np.float32)
    for g in range(_CORES):
        b, c = g // C, g % C
        # re-interleave the ACT/DVE half-chunk regions into original cols
        ydev = np.empty((_QROWS, _NCH, 2, 512), dtype=np.float16)
        ydev[:, :, 0, :] = np.asarray(
            res.results[g]["ya"]).reshape(_QROWS, _NCH, 512)
        ydev[:, :, 1, :] = np.asarray(
            res.results[g]["yb"]).reshape(_QROWS, _NCH, 512)
        z = ydev.reshape(7, 16, 64, 64, 4)
        z = z.transpose(0, 2, 3, 4, 1).astype(np.float32)  # o,mi,mj,ml,kh
        z = z.reshape(7, 64, 64, 64)
        for s in range(8):
            out[b, :, :, :, s * C + c] = z[_OMAP[s]]
    return out


# revision 31
# speedup vs baseline: 1.2125x; 1.0411x over previous
"""Trainium2 Bass kernel for a separable 3D Haar DWT (nn_DWT3D).

Problem: x [2, 128, 128, 128, 4] fp32, A [128, 128] (orthonormal Haar
analysis filter bank, 2-tap stride-2). Output: subband concat
[2, 64, 64, 64, 32].

Strategy (8 NeuronCores):
- Data-parallel over (batch, channel): B*C = 8 independent [128,128,128]
  volume transforms, one per core. Host deinterleaves channels on the
  way in and assembles the subband concat on the way out (numpy).
- KEY TRICK: the host lays each volume out with partition index
  p = (i&1, j&1, k&1, (k>>1)&15) and free index f = (i>>1, j>>1, k>>5).
  ALL THREE Haar butterfly passes then act on the partition axis, so ONE
  128-wide PE matmul (butterfly_i (x) butterfly_j (x) butterfly_k (x)
  I_16, 8 nonzeros per row, h^3 folded in) performs the whole 3D
  transform. No vector-engine pass at all.
- fp16 everywhere off-chip (tolerance is 2e-2; fp16 keeps us ~1e-3):
  halves DMA bytes and runs the PE at 1 cycle/row instead of fp32's
  effective 8 (2 half-speed passes).
- The reference's subband concat never emits the (a=H, b=L, d=H) octant
  (its HLH slice equals HHH), so only 112 of 128 output rows are
  computed, drained, and DMA'd out.
- Per 2048-col chunk: SP issues in-DMA -> 4 PE matmuls (512 cols each,
  one PSUM bank) -> drain PSUM->SBUF fp16 split across ACT and DVE ->
  GpSimd (SWDGE) issues out-DMA.
"""

import numpy as np

_N = 128
_CORES = 8
_F = _N * _N  # 16384 free columns per volume
# input DMA widths (decoupled from compute chunks): few, fat transfers
# (4-8KB row descriptors) keep the queue deep; DMA count must stay low
# (~19): the DMA semaphore pool is small and recycles in issue order.
_INW = [1024, 4096, 4096, 4096, 2048, 1024]
# compute chunk width: per chunk one 512-col matmul into each of two
# 1-bank PSUM tiles (ACT-drained A half, DVE-drained B half)
_CW = 1024
_NCH = _F // _CW
# per-side output col counts per out-DMA group (accumulates 512/chunk);
# small first group starts the output stream early, small final groups
# shorten the drain tail
_OUTG = [1024, 2048, 2048, 2048, 1024]
_QROWS = 112  # 7 of 8 output octants * 16 kh rows

# Octants (lhi, lhj, lhk) in device output row order; (1, 0, 1) omitted.
_OCT = [(0, 0, 0), (1, 0, 0), (0, 1, 0), (1, 1, 0),
        (0, 0, 1), (0, 1, 1), (1, 1, 1)]
# reference band order LLL,LLH,LHL,LHH,HLL,HLH,HHL,HHH -> index into _OCT
_OMAP = [0, 1, 2, 3, 4, 6, 5, 6]

_cache = {}


def _build():
    import concourse.mybir as mybir
    from concourse import bacc
    from concourse.tile import TileContext

    nc = bacc.Bacc("TRN2", target_bir_lowering=False, debug=False,
                   num_devices=_CORES)
    f16 = mybir.dt.float16
    f32 = mybir.dt.float32
    v = nc.dram_tensor("v", [_N, _F], f16, kind="ExternalInput")
    w = nc.dram_tensor("w", [_N, _QROWS], f16, kind="ExternalInput")
    # two output regions: ACT-drained (even 512-col banks) and DVE-drained
    # (odd banks); host re-interleaves
    ya_d = nc.dram_tensor("ya", [_QROWS, _F // 2], f16, kind="ExternalOutput")
    yb_d = nc.dram_tensor("yb", [_QROWS, _F // 2], f16, kind="ExternalOutput")

    with TileContext(nc) as tc:
        with (
            tc.tile_pool(name="wpool", bufs=1) as wpool,
            tc.tile_pool(name="vin", bufs=1) as vpool,
            tc.tile_pool(name="ya", bufs=1) as yapool,
            tc.tile_pool(name="yb", bufs=1) as ybpool,
            tc.tile_pool(name="psa", bufs=2, space="PSUM") as psapool,
            tc.tile_pool(name="psb", bufs=4, space="PSUM") as psbpool,
        ):
            wt = wpool.tile([_N, _QROWS], f16)
            nc.scalar.dma_start(out=wt[:], in_=w[:])

            vin = vpool.tile([_N, _F], f16)
            yat = yapool.tile([_QROWS, _F // 2], f16)
            ybt = ybpool.tile([_QROWS, _F // 2], f16)

            ioff = 0
            for iw in _INW:
                nc.sync.dma_start(out=vin[:, ioff:ioff + iw],
                                  in_=v[:, ioff:ioff + iw])
                ioff += iw

            gends = np.cumsum(_OUTG)
            gi = 0
            hmark = 0  # start of the not-yet-DMA'd output region
            for p in range(_NCH // 2):
                # A-halves of a chunk pair share one 2-bank PSUM tile so a
                # single ACT drains 1024 cols (halves the ACT instruction
                # overhead); B-halves use fine 1-bank tiles with 4 bufs so
                # the PE never stalls on drain recycling
                psa = psapool.tile([_QROWS, 1024], f32, tag="psa")
                for j in range(2):
                    k = 2 * p + j
                    off = k * _CW
                    ha = _CW // 2
                    psb = psbpool.tile([_QROWS, 512], f32, tag="psb")
                    nc.tensor.matmul(psa[:, j * 512:(j + 1) * 512], wt[:],
                                     vin[:, off:off + 512],
                                     start=True, stop=True)
                    nc.tensor.matmul(psb[:], wt[:],
                                     vin[:, off + ha:off + ha + 512],
                                     start=True, stop=True)
                    nc.vector.tensor_copy(ybt[:, k * 512:(k + 1) * 512],
                                          psb[:])
                h = p * 1024
                nc.scalar.copy(out=yat[:, h:h + 1024], in_=psa[:])
                h += 1024

                # grouped out-DMAs on the Scalar HWDGE ring (Sync would
                # queue them behind ALL input in the same FIFO ring; gpsimd
                # SWDGE has ~3us first-byte latency and a slower stream)
                if gi < len(gends) and h == gends[gi]:
                    he = gends[gi]
                    nc.scalar.dma_start(out=ya_d[:, hmark:he],
                                        in_=yat[:, hmark:he])
                    nc.scalar.dma_start(out=yb_d[:, hmark:he],
                                        in_=ybt[:, hmark:he])
                    hmark = he
                    gi += 1

    nc.compile()
    return nc


def _get_nc():
    if "nc" not in _cache:
        _cache["nc"] = _build()
    return _cache["nc"]


def _haar_structure_ok(A):
    """A must be the 2-tap stride-2 filter bank with taps (h, h) lowpass /
    (-h, h) highpass, which is what the butterflies hardcode."""
    if A.shape != (_N, _N):
        return False
    h = A[0, 0]
    if not np.isfinite(h) or abs(h) < 1e-8:
        return False
    expect = np.zeros((_N, _N), dtype=np.float32)
    for i in range(_N // 2):
        expect[i, 2 * i] = h
        expect[i, 2 * i + 1] = h
        expect[_N // 2 + i, 2 * i] = -h
        expect[_N // 2 + i, 2 * i + 1] = h
    return bool(np.allclose(A, expect, rtol=1e-5, atol=1e-7))


def _reference_host(x, A):
    """Generic numpy fallback (slow) for non-Haar A."""
    y = np.einsum("ai,nijkc->najkc", A, x, optimize=True)
    y = np.einsum("bj,najkc->nabkc", A, y, optimize=True)
    y = np.einsum("dk,nabkc->nabdc", A, y, optimize=True)
    return np.moveaxis(y, -1, 1)


def _assemble(y_full, B, C):
    """Slice transformed volumes y_full [B, C, 128,128,128] into the
    reference's subband concat [B, 64, 64, 64, 8*C] (incl. the duplicated
    HHH octant the reference produces)."""
    L, H = slice(0, 64), slice(64, 128)
    bands = [(L, L, L), (H, L, L), (L, H, L), (H, H, L),
             (L, L, H), (H, H, H), (L, H, H), (H, H, H)]
    out = np.empty((B, 64, 64, 64, 8 * C), dtype=np.float32)
    for s, (sa, sb, sd) in enumerate(bands):
        out[..., s * C:(s + 1) * C] = np.moveaxis(y_full[:, :, sa, sb, sd], 1, -1)
    return out


def kernel(x, A):
    from concourse.bass_utils import run_bass_kernel_spmd

    x = np.asarray(x, dtype=np.float32)
    A = np.asarray(A, dtype=np.float32)
    B, _, _, _, C = x.shape
    assert (B, C) == (2, 4) and x.shape[1:4] == (_N, _N, _N)

    if not _haar_structure_ok(A):
        return _assemble(_reference_host(x, A), B, C)

    h = float(A[0, 0])
    # W maps partition (ei, ej, ek, kh) -> (octant o, kh): all three
    # butterfly passes at once, h^3 folded in. lhsT = W.T [128, 112].
    sgn = np.array([[1.0, 1.0], [-1.0, 1.0]], dtype=np.float32)
    Wm = np.zeros((_QROWS, _N), dtype=np.float32)
    h3 = np.float32(h ** 3)
    for o, (lhi, lhj, lhk) in enumerate(_OCT):
        for ei in range(2):
            for ej in range(2):
                for ek in range(2):
                    c = h3 * sgn[lhi, ei] * sgn[lhj, ej] * sgn[lhk, ek]
                    for kh in range(16):
                        Wm[o * 16 + kh,
                           ei * 64 + ej * 32 + ek * 16 + kh] = c
    wT = np.ascontiguousarray(Wm.T.astype(np.float16))

    # Host layout: p = (i&1, j&1, k&1, (k>>1)&15), f = (i>>1, j>>1, k>>5)
    xs = np.transpose(x, (0, 4, 1, 2, 3))               # [B, C, i, j, k]
    t = xs.reshape(_CORES, 64, 2, 64, 2, 4, 16, 2)      # g,mi,ei,mj,ej,ml,kh,ek
    t = t.transpose(0, 2, 4, 7, 6, 1, 3, 5)             # g,ei,ej,ek,kh,mi,mj,ml
    V = np.ascontiguousarray(t).astype(np.float16).reshape(_CORES, _N, _F)

    in_maps = [{"v": V[g], "w": wT} for g in range(_CORES)]
    nc = _get_nc()
    res = run_bass_kernel_spmd(nc, in_maps, list(range(_CORES)))

    # Device rows (o, kh), cols (mi, mj, ml). Within-octant output index:
    # a = 64*lhi + mi, b = 64*lhj + mj, d = 64*lhk + 16*ml + kh.
    out = np.empty((B, 64, 64, 64, 8 * C), dtype=np.float32)
    for g in range(_CORES):
        b, c = g // C, g % C
        # re-interleave the ACT/DVE half-chunk regions into original cols
        ydev = np.empty((_QROWS, _NCH, 2, 512), dtype=np.float16)
        ydev[:, :, 0, :] = np.asarray(
            res.results[g]["ya"]).reshape(_QROWS, _NCH, 512)
        ydev[:, :, 1, :] = np.asarray(
            res.results[g]["yb"]).reshape(_QROWS, _NCH, 512)
        z = ydev.reshape(7, 16, 64, 64, 4)
        z = z.transpose(0, 2, 3, 4, 1).astype(np.float32)  # o,mi,mj,ml,kh
        z = z.reshape(7, 64, 64, 64)
        for s in range(8):
            out[b, :, :, :, s * C + c] = z[_OMAP[s]]
    return out
